# revision 20
# baseline (speedup 1.0000x reference)
"""Trainium2 Bass kernel for single-head attention + output projection + residual.

Math per batch element b (N=2048, D=512, U=128):
    Q = x @ W_q; K = x @ W_k; V = x @ W_v
    S = Q @ K.T / sqrt(U); A = softmax(S, axis=-1)
    out = (A @ V) @ W_o + b_o + x

Distribution: data-parallel over batch - 8 batch elements, one per NeuronCore.

v5 design (on top of v4):
- exp split across TWO engines: ScalarE keeps the table exp for ~9/16 key
  blocks; VectorE does the rest with a one-instruction Schraudolph exp:
  uint8_saturate(A8*s + B8) whose bits ARE the fp8e4m3 of exp(s) (the f32->u8
  cvt is round-to-nearest + saturating at 0, verified on HW). This halves the
  36us serial EXP wall that dominated v4.
- Q/K projections and V in fp8 DoubleRow (2 d-chunks per instruction).
- phase interleave: ctx-h0 + den-h1 matmuls ride inside the scores-h1 stream
  so the PE never idles long enough for HAM to re-throttle; den-h0 finalized
  at end of phase A; epilogues split Vector / Scalar+GpSimd.
- startup: xT ns0 split across two DMA queues so the first DR projection
  waits on 128KB, not 256KB.

Softmax max-subtraction is skipped: scores/sqrt(U) are bounded (~+-6) for any
well-scaled input; exp output with -2.5 shift stays under fp8e4's 240 ceiling.
"""

import numpy as np
import ml_dtypes

import concourse.bass as bass
import concourse.tile as tile
from concourse import bacc, mybir
from concourse.bass_utils import run_bass_kernel_spmd

N = 2048
D = 512
U = 128
NB = N // 128  # 16 query/key blocks
DC = D // 128  # 4 d-chunks
NS = N // 512  # 4 free-dim slices of 512
HQ = N // 2  # queries per half
NP = NB // 2  # 8 key-block pairs
X_SCALE = 32.0
W_SCALE = 256.0
QK_SCALE = X_SCALE * W_SCALE  # each of Q,K carries this factor
INV_SQRT_U = 1.0 / np.sqrt(U)
EXP_SHIFT = -2.5  # exp(s-2.5): cancels in normalization; keeps max logits
# (~5.7 + fp8 quant noise) safely under fp8e4's 240 ceiling (overflow = NaN)
SCL = INV_SQRT_U / (QK_SCALE * QK_SCALE)  # raw PSUM score -> logit
# Schraudolph fp8e4m3 exp: bits(exp(l)) ~ A8*l + B8 (sigma centers the error)
A8 = 8.0 / np.log(2.0)
B8 = 8.0 * (7.0 - 0.0564) + A8 * EXP_SHIFT

F32 = mybir.dt.float32
BF16 = mybir.dt.bfloat16
FP8 = mybir.dt.float8e4
U8 = mybir.dt.uint8

DR = mybir.MatmulPerfMode.DoubleRow


def build_attention_nc():
    nc = bacc.Bacc("TRN2", target_bir_lowering=False, debug=False)

    # xb pre-transposed on host -> one contiguous [128, 16KB-line] DMA
    xb_ext = nc.declare_dram_parameter("xb", [128, NB * D], BF16, isOutput=False)
    # wkqv (k|q|v packed, fp8*256) prepended to xT ([p, ns, c, n5] fp8*32):
    # one tensor so the startup DMA moves 2.5KB+ lines, not 512B lines
    xa_ext = nc.declare_dram_parameter(
        "xa", [128, 3 * D + NS * DC * 512], FP8, isOutput=False
    )
    wo_ext = nc.declare_dram_parameter("wo", [U, D], BF16, isOutput=False)
    # out transposed [p, nb, d]; host un-permutes. 4-qb batched stores
    # -> 4KB DMA lines instead of 16 stores of 1KB lines.
    out_ext = nc.declare_dram_parameter("out", [128, NB * D], BF16, isOutput=True)

    with tile.TileContext(nc) as tc:
        _build_body(nc, tc, xb_ext, xa_ext, wo_ext, out_ext)
    nc.compile()
    return nc


def _build_body(nc, tc, xb_ext, xa_ext, wo_ext, out_ext):
    from contextlib import ExitStack

    with ExitStack() as ctx:
        const = ctx.enter_context(tc.tile_pool(name="const", bufs=1))

        # ---- loads: one big tile [wkqv | xT]; 3 big-line DMA pieces ----
        W3 = 3 * D  # 1536 fp8 of packed weights ahead of xT
        xa_sb = const.tile([128, W3 + NS * DC * 512], FP8)
        wkqv_sb = xa_sb[:, 0:W3].rearrange("p (k g t u) -> p k g t u", k=3, g=2, t=2)
        wk_sb = wkqv_sb[:, 0]
        wq_sb = wkqv_sb[:, 1]
        wv_sb = wkqv_sb[:, 2]
        xT_sb = xa_sb[:, W3:].rearrange("p (ns c n) -> p ns c n", ns=NS, c=DC)
        wo_sb = const.tile([U, D], BF16)
        xb_sb = const.tile([128, NB, D], BF16)

        xa = xa_ext.ap()
        xb_r = xb_ext.ap().rearrange("p (nb d) -> p nb d", nb=NB)

        # piece 0: weights + ns0 c01 (2.5KB lines) gates the first matmuls;
        # xb/wo triggers are issued later from the scalar stream. The scalar
        # (Activation) queue also carries the exp table load first.
        nc.sync.dma_start(xa_sb[:, 0:W3 + 2048], xa[:, 0:W3 + 2048])
        nc.gpsimd.dma_start(
            xa_sb[:, W3 + 2048:W3 + 4096], xa[:, W3 + 2048:W3 + 4096]
        )
        nc.sync.dma_start(xa_sb[:, W3 + 4096:], xa[:, W3 + 4096:])

        ones8_sb = const.tile([128, 2, 32], FP8)
        nc.vector.memset(ones8_sb[:], 1.0)
        ident_sb = const.tile([1, 1], F32)
        nc.vector.memset(ident_sb[:], 1.0)
        eshift_sb = const.tile([128, 1], F32)
        nc.vector.memset(eshift_sb[:], EXP_SHIFT)
        junk_sb = const.tile([128, 512], FP8)
        nc.vector.memset(junk_sb[:], 1.0)
        # force the exp activation table load while DMAs are in flight
        scratch = const.tile([128, 1], F32)
        nc.scalar.activation(
            scratch[:], eshift_sb[:], mybir.ActivationFunctionType.Exp, scale=1.0
        )

        QT_sb = const.tile([U, N], BF16)
        KT_sb = const.tile([U, N], BF16)
        V_sb = const.tile([128, NB, U], FP8)  # kb-pair p at [:, 2p:2p+2, :]
        ctxT_sb = const.tile([U, N], BF16)
        den_sb = [const.tile([1, HQ], F32, name=f"den_sb_{h}") for h in range(2)]
        r_sb = const.tile([128, NB], F32)

        # den accumulators: DR matmul dst must sit at partition base 0. Tiles
        # are full [128, 512] banks so the den transposes can reuse them
        # (cols 0:4) after the row copy - no extra PSUM tile needed.
        den_ps = {}
        e_t = [None] * NB  # paired e-tiles [128, 2, HQ], 8 per half
        ep = ctx.enter_context(tc.tile_pool(name="e_sb", bufs=16))

        def proj_slice(pool, w_sb, oT, ns, eng):
            # fp8 DoubleRow: 2 d-chunks per instruction (FD=512 >= 256)
            ps = pool.tile([128, 512], F32, tag="s", name=f"pp_{oT.tensor.name}_{ns}")
            for g in range(2):
                nc.tensor.matmul(
                    ps[:],
                    lhsT=w_sb[:, g],
                    rhs=xT_sb[:, ns, 2 * g:2 * g + 2, :],
                    start=(g == 0),
                    stop=(g == 1),
                    perf_mode=DR,
                )
            dst = oT[:, ns * 512:(ns + 1) * 512]
            if eng is nc.scalar:
                nc.scalar.copy(dst, ps[:])
            else:
                nc.vector.tensor_copy(dst, ps[:])

        def make_v(pool, g4, eng):
            # 4 key-blocks' V projections into one PSUM tile (DR over
            # d-chunks), one rescaled fp8 copy out (V keeps x*32 -> mul 1/256)
            ps = pool.tile([128, 512], F32, tag="s", name=f"v_{g4}")
            for q in range(4):
                kb = g4 * 4 + q
                ns, n0 = divmod(kb * 128, 512)
                for g in range(2):
                    nc.tensor.matmul(
                        ps[:, q * 128:(q + 1) * 128],
                        lhsT=xT_sb[:, ns, 2 * g:2 * g + 2, n0:n0 + 128],
                        rhs=wv_sb[:, g],
                        start=(g == 0),
                        stop=(g == 1),
                        perf_mode=DR,
                        skip_group_check=True,
                    )
            dst = V_sb[:, g4 * 4:(g4 + 1) * 4, :]
            if eng is nc.scalar:
                nc.scalar.mul(dst, ps[:], 1.0 / W_SCALE)
            else:
                nc.vector.tensor_scalar(
                    dst, ps[:], 1.0 / W_SCALE, None, op0=mybir.AluOpType.mult
                )

        def den_mms(pool, h, pr):
            for j in range(2):
                if (h, j) not in den_ps:
                    den_ps[(h, j)] = pool.tile(
                        [128, 512], F32, tag="den", name=f"den_{h}_{j}"
                    )
                nc.tensor.matmul(
                    den_ps[(h, j)][0:32, :],
                    lhsT=ones8_sb[:],
                    rhs=e_t[h * NP + pr][:, :, j * 512:(j + 1) * 512],
                    start=(pr == 0),
                    stop=(pr == NP - 1),
                    perf_mode=DR,
                    skip_group_check=True,
                )

        def den_rows(h):
            # den rows -> SBUF (split across engines)
            nc.scalar.copy(den_sb[h][:, 0:512], den_ps[(h, 0)][0:1, :])
            nc.vector.tensor_copy(den_sb[h][:, 512:1024], den_ps[(h, 1)][0:1, :])

        def den_transposes(h):
            # 8 PE transposes back into the (already-copied) den banks
            for i in range(8):
                nc.tensor.matmul(
                    den_ps[(h, i // 4)][:, i % 4:i % 4 + 1],
                    lhsT=den_sb[h][:, i * 128:(i + 1) * 128],
                    rhs=ident_sb[:],
                    is_transpose=True,
                    skip_group_check=True,
                )

        def den_recip(h):
            nc.vector.reciprocal(r_sb[:, h * 8:h * 8 + 4], den_ps[(h, 0)][:, 0:4])
            nc.vector.reciprocal(
                r_sb[:, h * 8 + 4:h * 8 + 8], den_ps[(h, 1)][:, 0:4]
            )

        def scores_block(sp, h, kb):
            pr = (h * NB + kb) // 2
            t = kb % 2
            q0 = h * HQ
            if t == 0:
                e_t[pr] = ep.tile([128, 2, HQ], FP8, tag="e", name=f"e_{h}_{kb}")
            s_ps = sp.tile([128, HQ], F32, tag="s", name=f"s_{h}_{kb}")
            for j in range(2):
                nc.tensor.matmul(
                    s_ps[:, j * 512:(j + 1) * 512],
                    lhsT=KT_sb[:, kb * 128:(kb + 1) * 128],
                    rhs=QT_sb[:, q0 + j * 512:q0 + (j + 1) * 512],
                    start=True,
                    stop=True,
                )
            if kb % 2 == 1 and kb != 15:
                # VectorE Schraudolph exp: u8 bits of A8*logit+B8 ARE the fp8
                nc.vector.tensor_scalar(
                    e_t[pr][:, t, :].bitcast(U8),
                    s_ps[:],
                    A8 * SCL,
                    B8,
                    op0=mybir.AluOpType.mult,
                    op1=mybir.AluOpType.add,
                )
            else:
                nc.scalar.activation(
                    e_t[pr][:, t, :],
                    s_ps[:],
                    mybir.ActivationFunctionType.Exp,
                    bias=eshift_sb[:],
                    scale=SCL,
                )

        def ctx_mms(cp_tiles, h, pair):
            pr = h * NP + pair
            v2 = V_sb[:, 2 * pair:2 * pair + 2, :]
            for j in range(2):
                nc.tensor.matmul(
                    cp_tiles[j][:],
                    lhsT=v2,
                    rhs=e_t[pr][:, :, j * 512:(j + 1) * 512],
                    start=(pair == 0),
                    stop=(pair == NP - 1),
                    perf_mode=DR,
                    skip_group_check=True,
                )

        def ctx_copy(cp_tiles, h, j, eng):
            dst = ctxT_sb[:, h * HQ + j * 512:h * HQ + (j + 1) * 512]
            if eng is nc.scalar:
                nc.scalar.copy(dst, cp_tiles[j][:])
            else:
                nc.vector.tensor_copy(dst, cp_tiles[j][:])

        # ---- phases A/B: one PSUM ring carries proj + scores + V tiles ----
        # ring bufs=3 (6 banks); the remaining 2 banks are time-shared by
        # scoped pools: den-h0 (A tail) -> ctx-h0 (B first half) -> y-h0
        # (B second half). The whole h0 epilogue rides inside the h1 scores
        # stream; phase C only finishes the h1 half.
        op = ctx.enter_context(tc.tile_pool(name="o_sb", bufs=3))
        o4 = {}
        out_r = out_ext.ap().rearrange("p (nb d) -> p nb d", nb=NB)

        def epilogue_qb(dp, h, qb_local, store_eng, via_scalar=False):
            qb = h * 8 + qb_local
            g, sl = divmod(qb, 4)
            if sl == 0:
                o4[g] = op.tile([128, 4, D], BF16, tag="o", name=f"o4_{g}")
            y_ps = dp.tile([128, D], F32, tag="d", name=f"y_{qb}")
            nc.tensor.matmul(
                y_ps[:],
                lhsT=ctxT_sb[:, qb * 128:(qb + 1) * 128],
                rhs=wo_sb[:],
                start=True,
                stop=True,
                skip_group_check=True,
            )
            o_t = o4[g][:, sl, :]
            if via_scalar:
                # ScalarE does y*r (per-partition scale), GpSimd adds the
                # residual (SBUF-only)
                nc.scalar.mul(o_t, y_ps[:], r_sb[:, qb:qb + 1])
                nc.gpsimd.tensor_add(o_t, o_t, xb_sb[:, qb, :])
            else:
                nc.vector.scalar_tensor_tensor(
                    o_t,
                    in0=y_ps[:],
                    scalar=r_sb[:, qb:qb + 1],
                    in1=xb_sb[:, qb, :],
                    op0=mybir.AluOpType.mult,
                    op1=mybir.AluOpType.add,
                )
            if sl == 3:
                store_eng.dma_start(out_r[:, 4 * g:4 * g + 4, :], o4[g][:])

        with tc.tile_pool(name="ring_ps", bufs=3, space="PSUM") as sp:
            # HAM warmup: junk matmuls keep the PE busy while the first DMAs
            # land so real matmuls start at 2.4 GHz.
            warm = sp.tile([128, 512], F32, tag="s", name="warm")
            for i in range(8):
                nc.tensor.matmul(
                    warm[:], lhsT=junk_sb[:, 0:128], rhs=junk_sb[:],
                    start=True, stop=True, skip_group_check=True,
                )

            proj_slice(sp, wk_sb, KT_sb, 0, nc.scalar)
            proj_slice(sp, wq_sb, QT_sb, 0, nc.vector)
            proj_slice(sp, wq_sb, QT_sb, 1, nc.scalar)

            for kb in range(4):
                scores_block(sp, 0, kb)
            proj_slice(sp, wk_sb, KT_sb, 1, nc.vector)
            for kb in range(4, 8):
                scores_block(sp, 0, kb)
                if kb == 4:
                    nc.scalar.dma_start(wo_sb[:], wo_ext.ap())
                if kb == 6:
                    nc.scalar.dma_start(xb_sb[:, 0:8], xb_r[:, 0:8])
            proj_slice(sp, wk_sb, KT_sb, 2, nc.scalar)
            for kb in range(8, 12):
                scores_block(sp, 0, kb)
                if kb == 8:
                    nc.scalar.dma_start(xb_sb[:, 8:16], xb_r[:, 8:16])
            proj_slice(sp, wk_sb, KT_sb, 3, nc.vector)
            for kb in range(12, 16):
                scores_block(sp, 0, kb)

            proj_slice(sp, wq_sb, QT_sb, 2, nc.vector)
            proj_slice(sp, wq_sb, QT_sb, 3, nc.scalar)
            for g4 in range(4):
                make_v(sp, g4, nc.scalar if g4 % 2 == 0 else nc.vector)

            # den-h0 burst + finalize (all e-h0 ready; exp chases scores-h0)
            with tc.tile_pool(name="den0_ps", bufs=2, space="PSUM") as den0:
                for pr in range(NP):
                    den_mms(den0, 0, pr)
                den_rows(0)
                den_transposes(0)
                den_recip(0)

            # h1 scores first half, with the full ctx-h0 accumulation
            # interleaved (one pair per kb)
            with tc.tile_pool(name="ctx0_ps", bufs=2, space="PSUM") as cp0:
                ctx0 = [
                    cp0.tile([U, 512], F32, tag="ctx", name=f"ctx_ps_0_{j}")
                    for j in range(2)
                ]
                for kb in range(8):
                    scores_block(sp, 1, kb)
                    ctx_mms(ctx0, 0, kb)
                ctx_copy(ctx0, 0, 0, nc.scalar)
                ctx_copy(ctx0, 0, 1, nc.vector)

            # h1 scores second half, with the h0 epilogues interleaved
            with tc.tile_pool(name="y0_ps", bufs=2, space="PSUM") as dp_b:
                for kb in range(8, 16):
                    scores_block(sp, 1, kb)
                    epilogue_qb(
                        dp_b, 0, kb - 8, nc.sync, via_scalar=(kb % 2 == 1)
                    )

        # ---- phase C: den-h1 + ctx-h1 + h1 epilogues ----
        with (
            tc.tile_pool(name="den1_ps", bufs=2, space="PSUM") as den1,
            tc.tile_pool(name="ctx1_ps", bufs=2, space="PSUM") as cp1,
            tc.tile_pool(name="y1_ps", bufs=3, space="PSUM") as dp_c,
        ):
            ctx1 = [
                cp1.tile([U, 512], F32, tag="ctx", name=f"ctx_ps_1_{j}")
                for j in range(2)
            ]
            # den-h1 chain runs first and uninterrupted: pair 7 waits only
            # the kb15 exp, so r-h1 is ready before the first h1 epilogue
            for pr in range(NP):
                den_mms(den1, 1, pr)
            den_rows(1)
            den_transposes(1)
            den_recip(1)
            for pair in range(NP):
                ctx_mms(ctx1, 1, pair)
            ctx_copy(ctx1, 1, 0, nc.scalar)
            ctx_copy(ctx1, 1, 1, nc.vector)
            # slow scalar+gpsimd paths early, single-op vector paths last
            for qb_local in range(8):
                epilogue_qb(
                    dp_c, 1, qb_local,
                    nc.sync,
                    via_scalar=(qb_local in (0, 2, 4)),
                )


_NC_CACHE = {}


def _get_nc():
    if "nc" not in _NC_CACHE:
        _NC_CACHE["nc"] = build_attention_nc()
    return _NC_CACHE["nc"]


def prep_in_maps(inputs, W_q, W_k, W_v, W_o, b_o):
    """Host-side sharding + layout prep. One batch element per core."""
    B = inputs.shape[0]
    bf = ml_dtypes.bfloat16
    f8 = ml_dtypes.float8_e4m3

    def rearr_w8(w):  # [D, U] -> [128, (g t u)] fp8*W_SCALE, d = (2g+t)*128+p
        w8 = (np.asarray(w) * W_SCALE).astype(f8)
        return np.ascontiguousarray(
            w8.reshape(DC // 2, 2, 128, U).transpose(2, 0, 1, 3).reshape(128, D)
        )

    wkqv_r = np.stack(
        [rearr_w8(W_k), rearr_w8(W_q), rearr_w8(W_v)], axis=1
    ).reshape(128, 3 * D)
    # fold the V-side residual scale (1/X_SCALE) into W_o
    wo_r = np.ascontiguousarray(np.asarray(W_o) / X_SCALE).astype(bf)
    bo = np.asarray(b_o, dtype=np.float32)

    in_maps = []
    for b in range(B):
        xf = np.asarray(inputs[b], dtype=np.float32)
        # xT fp8 host layout [p, ns, c, n5], scaled by X_SCALE
        xT4 = (
            (xf.T * X_SCALE).astype(f8)
            .reshape(DC, 128, NS, 512)
            .transpose(1, 2, 0, 3)
            .reshape(128, NS * DC * 512)
        )
        xbt = (
            (xf + bo).astype(bf)
            .reshape(NB, 128, D)
            .transpose(1, 0, 2)
            .reshape(128, NB * D)
        )
        in_maps.append({
            "xb": np.ascontiguousarray(xbt),
            "xa": np.ascontiguousarray(np.concatenate([wkqv_r, xT4], axis=1)),
            "wo": wo_r,
        })
    return in_maps


def kernel(inputs, W_q, W_k, W_v, W_o, b_o):
    in_maps = prep_in_maps(inputs, W_q, W_k, W_v, W_o, b_o)
    nc = _get_nc()
    res = run_bass_kernel_spmd(nc, in_maps, core_ids=list(range(len(in_maps))))
    return np.stack(
        [
            res.results[i]["out"]
            .astype(np.float32)
            .reshape(128, NB, D)
            .transpose(1, 0, 2)
            .reshape(N, D)
            for i in range(len(in_maps))
        ],
        axis=0,
    )
